# revision 1
# baseline (speedup 1.0000x reference)
"""Trainium2 Bass kernel for the light-field disparity cost-volume build.

Input  x:   (2, 16, 25, 128, 128) f32  (b, c, n=angRes^2, h, w)
Output:     (2, 16, 25, 9, 128, 128) f32  (b, c, n, D, h, w)

out[b,c,(a1,a2),d,y,x] = x[b,c,(a1,a2), y + d*(2-a1), x + d*(2-a2)]
(zero outside the image), d in [-4, 4].

Pure data movement. Sharding: the 32 (b*c) slices split 4-per-core over
8 NeuronCores (data parallel, no cross-core communication).

Strategy (big-descriptor): the baseline paid one 512B DMA descriptor
per shifted output row (~80k descriptors/core), which caps the HWDGE
rings at ~44 GB/s each. Here the column shift (the thing that breaks
DRAM-side row contiguity) is done by the DVE as an SBUF->SBUF strided
tensor_copy, so every output tile is materialized in SBUF with 32
consecutive output rows per partition; stores then use 16 KB
descriptors and the row shift is absorbed into the store's DRAM offset
(any run of rows of a tile is contiguous in DRAM). Four staging slots
deepen the DVE->store pipeline; a2==2 edges and zero bands ride SWDGE
so the HWDGE-pinned engines 0-3 only carry the interior edge stores.
Baseline 540us -> 365us.

Measured HW behavior that shaped the work split below:
  - HWDGE (sync/scalar rings): each dma_start's descriptors are dealt
    to SDMA engines in chunks of ~4 (16KB descs; ~16 for 1KB descs),
    restarting at engine 0 for EVERY DMA -> a <=16-descriptor ring DMA
    only ever uses engines 0-3. Ring issue cost ~0.6us + ~13ns/desc.
  - SWDGE (gpsimd): descriptors round-robin continuously over all 16
    SDMA engines regardless of DMA size, ~0.7us/dma_start on the Q7.
  - Hence: the bulk (full 4-partition-group stores, 12-16 x 16KB descs
    each) goes on gpsimd/SWDGE; the rings carry only the small edge
    stores (split into 1KB descriptors) and zero bands.
  - max_dma_last_dim is in BYTES.

Layout (per partition free dim, f32 elems):
  [0 .. 5*4096)    RAW: a2-major columns. Column a2 (4096 elems) holds
                   views (a1, a2) for all a1, s on partitions
                   p = 16*a1 + 4*g + s, each partition = 32 consecutive
                   image rows (g-th row group) of that (view, slice).
  [RAW .. +2*4096) STAGING: 2 slots, same partition map as a RAW
                   column; slot holds the column-shifted copy for one
                   (d, a2) combo, margins zeroed.
  [ZOFF .. +1024)  zeros for the zero-row bands.

Work split:
  DVE     column-shift copies (u16-bitcast tensor_copy, 4x mode) +
          margin memsets, one combo (d != 0, a2 != 2) at a time.
  gpsimd  loads (25 view DMAs); d=0 tiles (one DRAM->DRAM copy); per
          combo the 5 full-group stores from staging; a2==2 (c==0)
          full-group stores straight from RAW (row shift in the store
          offset).
  sync    per combo, edge stores (the partial row group at the shifted
          boundary) for a1 {0,1}; half the a2==2 edges + zero bands.
  scalar  same for a1 {3,4} and the other half.
"""

from contextlib import ExitStack

import numpy as np

import concourse.bass as bass
import concourse.mybir as mybir
from concourse.bass import AP
from concourse.bass_utils import run_bass_kernel_spmd

F32 = mybir.dt.float32
U16 = mybir.dt.uint16

B, C, NV, H, W = 2, 16, 25, 128, 128
A = 5
MIND, MAXD = -4, 4
D = MAXD - MIND + 1
NCORES = 8
NS = (B * C) // NCORES      # slices per core = 4

RPP = 32                    # image rows per partition
G = H // RPP                # row groups per tile = 4
FREE = RPP * W              # elems per partition per (view, slice) = 4096

X_V = H * W                 # input view stride (elems)
X_S = NV * X_V              # input slice stride
O_T = H * W                 # output tile stride
O_V = D * O_T               # output view stride
O_S = NV * O_V              # output slice stride

SOFF = A * FREE             # staging offset (after 5 RAW columns)
NSLOT = 4                   # staging slots (pipeline depth)
ZOFF = SOFF + NSLOT * FREE  # zeros region offset
ZLEN = 1024
PITCH = ZOFF + ZLEN

D_LIST = [d for d in range(MIND, MAXD + 1) if d != 0]
CORD = [0, 1, 3, 4]         # a2 columns with c != 0, in load order
COMBOS = [(a2, d) for a2 in CORD for d in D_LIST]   # 32 combos


def _p0(a1, g=0, s=0):
    """Partition index of (a1, g, s) within a column (g-major, s-minor).

    SBUF DMA APs must keep the partition dim as dim 0 with stride of one
    partition, so every DMA below addresses a DENSE partition range;
    iteration order over partitions is (g outer, s inner).
    """
    return 16 * a1 + 4 * g + s


def _build_nc():
    nc = bass.Bass()
    x = nc.dram_tensor("x", [NS, NV, H, W], F32, kind="ExternalInput")
    out = nc.dram_tensor("out", [NS, NV, D, H, W], F32, kind="ExternalOutput")

    # zero-band jobs (d, a1) with r != 0; batched over (a2, s) in one DMA
    zjobs = [
        (d, a1)
        for a1 in range(A)
        for d in D_LIST
        if d * (A // 2 - a1) != 0
    ]

    with (
        ExitStack() as stack,
        nc.sbuf_tensor([128, PITCH], F32) as buf,
        nc.semaphore("vsem") as vsem,     # staged combos (1/combo, DVE-ordered)
        nc.semaphore("zsem") as zsem,     # zeros region ready
        nc.semaphore("gsem") as gsem,     # gpsimd store completions
        nc.Block() as block,
    ):
        # Waits on a DMA-completion semaphore are only safe at its full
        # running total (each dma_start's 16 increments land unordered
        # across SDMA engines). Hence: one sem per a2 column for loads
        # (waited at 80 = all 5 view loads), and one sem per (ring,
        # staging slot) for combo stores (waited at 64/80 per past use).
        lsc = [stack.enter_context(nc.semaphore(f"lsc{j}")) for j in range(A)]
        gf = [stack.enter_context(nc.semaphore(f"gf{i}")) for i in range(NSLOT)]
        eA = [stack.enter_context(nc.semaphore(f"eA{i}")) for i in range(NSLOT)]
        eB = [stack.enter_context(nc.semaphore(f"eB{i}")) for i in range(NSLOT)]

        @block.vector
        def _(vector):
            vector.memset(AP(buf, ZOFF, [[PITCH, 128], [1, ZLEN]]), 0.0)\
                .then_inc(zsem, 1)
            cur_col = None
            for k, (a2, d) in enumerate(COMBOS):
                c = d * (A // 2 - a2)
                if a2 != cur_col:
                    cur_col = a2
                    vector.wait_ge(lsc[a2], 80)
                if k >= NSLOT:
                    # slot k%NSLOT was last used by combo k-NSLOT; wait
                    # for its stores: 5 fulls (pool) + 2+2 edges (rings)
                    vector.wait_ge(gf[k % NSLOT], 80 * (k // NSLOT))
                    vector.wait_ge(eA[k % NSLOT], 32 * (k // NSLOT))
                    vector.wait_ge(eB[k % NSLOT], 32 * (k // NSLOT))
                so = SOFF + (k % NSLOT) * FREE
                n = W - abs(c)
                src_off = a2 * FREE + max(c, 0)
                dst_off = so + max(-c, 0)
                # u16 bitcast: 2x elem counts/strides, 4x DVE mode
                vector.tensor_copy(
                    out=AP(buf, dst_off, [[PITCH, 80], [W, RPP], [1, n]]
                           ).bitcast(U16),
                    in_=AP(buf, src_off, [[PITCH, 80], [W, RPP], [1, n]]
                           ).bitcast(U16),
                )
                m_off = so + (W - c if c > 0 else 0)
                vector.memset(
                    AP(buf, m_off, [[PITCH, 80], [W, RPP], [1, abs(c)]]), 0.0
                ).then_inc(vsem, 1)
            for i in range(NSLOT):
                vector.wait_ge(gf[i], 80 * (len(COMBOS) // NSLOT))
                vector.wait_ge(eA[i], 32 * (len(COMBOS) // NSLOT))
                vector.wait_ge(eB[i], 32 * (len(COMBOS) // NSLOT))

        def full_store(engine, a2, d, a1, sem, src_col_off, mdld=1024):
            """Full row-groups of tile (*, a1*5+a2, d) from SBUF.

            Partitions (g, s) for g in [g0, g0+ng) are dense; iteration
            is g outer, s inner on both sides. For HWDGE rings,
            max_dma_last_dim=1024 splits each partition's 16KB run into
            4KB descriptors so the HWDGE's chunk-of-4 engine dealing
            spreads the DMA over 12-16 SDMA engines instead of 3-4.
            SWDGE (gpsimd) round-robins whole descriptors over all 16
            engines, so it keeps 16KB descriptors (mdld=None).
            """
            r = d * (A // 2 - a1)
            g0 = 1 if r > 0 else 0
            ng = G if r == 0 else G - 1
            v = a1 * A + a2
            engine.dma_start(
                out=AP(out, v * O_V + (d - MIND) * O_T + (RPP * g0 - r) * W,
                       [[FREE, ng], [O_S, NS], [1, FREE]]),
                in_=AP(buf, _p0(a1, g0) * PITCH + src_col_off,
                       [[PITCH, 4 * ng], [1, FREE]]),
                max_dma_last_dim=mdld,
            ).then_inc(sem, 16)

        def edge_store(engine, a2, d, a1, sem, src_col_off, split=True):
            """Partial row-group at the shifted edge (r != 0 only)."""
            r = d * (A // 2 - a1)
            nr = RPP - abs(r)
            v = a1 * A + a2
            if r > 0:
                # group 0, input rows [r, 32) -> output rows [0, 32-r)
                src = _p0(a1, 0) * PITCH + src_col_off + r * W
                dst = v * O_V + (d - MIND) * O_T
            else:
                # group 3, input rows [96, 128-|r|) -> out [96+|r|, 128)
                src = _p0(a1, G - 1) * PITCH + src_col_off
                dst = v * O_V + (d - MIND) * O_T + (96 - r) * W
            # max_dma_last_dim is in BYTES: 1KB descriptors -> 48-62 per
            # DMA -> ~12-15 HWDGE chunks -> spread over most SDMA
            # engines. SWDGE (split=False): keep whole 12-15KB descs.
            engine.dma_start(
                out=AP(out, dst, [[O_S, NS], [1, nr * W]]),
                in_=AP(buf, src, [[PITCH, NS], [1, nr * W]]),
                max_dma_last_dim=1024 if split else None,
            ).then_inc(sem, 16)

        def zero_band(engine, d, a1, sem):
            r = d * (A // 2 - a1)
            dst = (a1 * A) * O_V + (d - MIND) * O_T + ((H - r) * W if r > 0 else 0)
            engine.dma_start(
                out=AP(out, dst, [[O_V, A], [O_S, NS], [1, abs(r) * W]]),
                in_=AP(buf, ZOFF, [[PITCH, A * NS], [1, abs(r) * W]]),
            ).then_inc(sem, 16)

        @block.sync
        def _(sync):
            # interior edge stores a1 {0,1} per combo (1KB descriptors)
            cur_col = None
            for k, (a2, d) in enumerate(COMBOS):
                if a2 != cur_col:
                    cur_col = a2
                    sync.wait_ge(lsc[a2], 80)
                sync.wait_ge(vsem, k + 1)
                so = SOFF + (k % NSLOT) * FREE
                for a1 in (0, 1):
                    edge_store(sync, a2, d, a1, eA[k % NSLOT], so)
            for i in range(NSLOT):
                sync.wait_ge(eA[i], 32 * (len(COMBOS) // NSLOT))

        @block.scalar
        def _(scalar):
            cur_col = None
            for k, (a2, d) in enumerate(COMBOS):
                if a2 != cur_col:
                    cur_col = a2
                    scalar.wait_ge(lsc[a2], 80)
                scalar.wait_ge(vsem, k + 1)
                so = SOFF + (k % NSLOT) * FREE
                for a1 in (3, 4):
                    edge_store(scalar, a2, d, a1, eB[k % NSLOT], so)
            for i in range(NSLOT):
                scalar.wait_ge(eB[i], 32 * (len(COMBOS) // NSLOT))

        @block.gpsimd
        def _(gpsimd):
            # loads: one DMA per view, a2-column order CORD + [2]
            for a2 in CORD + [2]:
                for a1 in range(A):
                    gpsimd.dma_start(
                        out=AP(buf, _p0(a1) * PITCH + a2 * FREE,
                               [[PITCH, 16], [1, FREE]]),
                        in_=AP(x, (a1 * A + a2) * X_V,
                               [[FREE, G], [X_S, NS], [1, FREE]]),
                    ).then_inc(lsc[a2], 16)
            # d=0 tiles: straight DRAM->DRAM copy of every view (one DMA,
            # big descriptors, SWDGE spreads them over all 16 engines)
            gpsimd.dma_start(
                out=AP(out, (0 - MIND) * O_T, [[O_V, NS * NV], [1, X_V]]),
                in_=AP(x, 0, [[X_V, NS * NV], [1, X_V]]),
                max_dma_last_dim=8192,
            ).then_inc(gsem, 16)
            n_misc = 1
            # per combo: the 5 full-group stores (16KB descriptors,
            # round-robined over all 16 SDMA engines), weaving in the
            # a2==2 fulls+edges (c==0, straight from RAW) and the zero
            # bands so everything overlaps the combo pipeline
            cur_col = None
            a22 = [(d, a1) for d in D_LIST for a1 in range(A)]
            a2e = [(d, a1) for d in D_LIST for a1 in range(A)
                   if d * (A // 2 - a1) != 0]
            for k, (a2, d) in enumerate(COMBOS):
                if a2 != cur_col:
                    cur_col = a2
                    gpsimd.wait_ge(lsc[a2], 80)
                gpsimd.wait_ge(vsem, k + 1)
                so = SOFF + (k % NSLOT) * FREE
                for a1 in range(A):
                    full_store(gpsimd, a2, d, a1, gf[k % NSLOT], so,
                               mdld=None)
                if k >= 12 and k - 12 < len(a22):
                    if k == 12:
                        gpsimd.wait_ge(lsc[2], 80)
                    dd, aa = a22[k - 12]
                    full_store(gpsimd, 2, dd, aa, gsem, 2 * FREE, mdld=None)
                    n_misc += 1
                    dd, aa = a22[k - 12 + 20]
                    full_store(gpsimd, 2, dd, aa, gsem, 2 * FREE, mdld=None)
                    n_misc += 1
                if k >= 12 and k - 12 < len(a2e):
                    dd, aa = a2e[k - 12]
                    edge_store(gpsimd, 2, dd, aa, gsem, 2 * FREE,
                               split=False)
                    n_misc += 1
                    if k - 12 + 20 < len(a2e):
                        dd, aa = a2e[k - 12 + 20]
                        edge_store(gpsimd, 2, dd, aa, gsem, 2 * FREE,
                                   split=False)
                        n_misc += 1
                if k == 0:
                    gpsimd.wait_ge(zsem, 1)
                if k < len(zjobs):
                    dz, az_ = zjobs[k]
                    zero_band(gpsimd, dz, az_, gsem)
                    n_misc += 1
            for i in range(NSLOT):
                gpsimd.wait_ge(gf[i], 80 * (len(COMBOS) // NSLOT))
            gpsimd.wait_ge(gsem, 16 * n_misc)

    return nc


_NC = None


def _get_nc():
    global _NC
    if _NC is None:
        _NC = _build_nc()
    return _NC


def kernel(x: np.ndarray) -> np.ndarray:
    assert x.shape == (B, C, NV, H, W), x.shape
    xs = np.ascontiguousarray(x.astype(np.float32, copy=False)).reshape(
        B * C, NV, H, W
    )
    in_maps = [{"x": xs[NS * k : NS * (k + 1)]} for k in range(NCORES)]
    res = run_bass_kernel_spmd(_get_nc(), in_maps, core_ids=list(range(NCORES)))
    out = np.concatenate([r["out"] for r in res.results], axis=0)
    return out.reshape(B, C, NV, D, H, W)



# revision 2
# speedup vs baseline: 1.8558x; 1.8558x over previous
"""Trainium2 Bass kernel for the light-field disparity cost-volume build.

Input  x:   (2, 16, 25, 128, 128) f32  (b, c, n=angRes^2, h, w)
Output:     (2, 16, 25, 9, 128, 128) f32  (b, c, n, D, h, w)

out[b,c,(a1,a2),d,y,x] = x[b,c,(a1,a2), y + d*(2-a1), x + d*(2-a2)]
(zero outside the image), d in [-4, 4].

Pure data movement; sharding: 32 (b*c) slices split 4-per-core over 8
NeuronCores (data parallel).

v4 strategy (indirect scatter of whole partition blocks): the previous
design issued ~420 regular dma_starts per core; the Q7 SWDGE costs ~1us
per dma_start (994ns fixed + 0.34ns/descriptor) and the HWDGE rings
deal all their descriptors to SDMA engines 0-3 only, so the gpsimd
issue rate (~227us) and the engine 0-3 hot spot (~330us busy) capped
the kernel at ~361us.

Key HW facts (probed):
  - indirect_dma_start supports exactly ONE block per SBUF partition
    (block = the per-partition contiguous run of in_, idx[p] gives the
    DRAM block index; multi-block-per-partition offset tables scramble).
  - Issue cost ~1us per instruction regardless of descriptor count, and
    SWDGE round-robins descriptors over all 16 SDMA engines.

Design: the output tensor is written PADDED ([TOTR, 128] rows; host
un-pads with a row gather). Each (slice s, view v, disparity d) tile
gets a start row S(t) chosen so S(t) == r (mod 32) where r = d*(2-a1)
is the tile's row shift, with >=16 pad rows between tiles. Then every
one of the tile's 4 row-groups - including the partial edge group - can
be stored as a full 16KB partition block at 4096-elem-aligned dest
(32*g - r + S(t) is a multiple of 32 rows): the <=8 overhanging rows
land in the inter-tile padding. So per (column a2, d-sign bank, d) ONE
indirect_dma_start scatters all 80 partition blocks (5 a1 x 4 g x 4 s),
fulls and edges alike, with no partial-group descriptors at all. d=0
tiles are scattered straight from RAW the same way (one instruction
per column; also kills the old DRAM->DRAM d0 copy's extra 6.5MB read).

Per core: 1 idx load + 25 view loads + 5 d0 + 40 bank-store indirect
instructions on gpsimd (~70us of Q7), 16 tiny zero-band DMAs per HWDGE
ring, all store descriptors 16KB and evenly spread - leaving the HBM
write floor (~59MB at ~358GB/s plus 6.5MB of reads) as the limiter.

The pad schedule is periodic in (a1, d) (r doesn't depend on a2 or s),
so view stride V_LEN(a1) and slice stride S_LEN are uniform and the
zero bands keep the baseline's batched [[a2 x s]] regular-DMA shape.

DVE staging: per column a2, two 4-slot banks (d<0, d>0) of
column-shifted copies (u16-bitcast tensor_copy + margin memsets; the
a2=2 column is a plain copy); banks double-buffer against their
stores; RAW columns rotate through a 3-slot ring so loads run 3
columns ahead.
"""

from contextlib import ExitStack

import numpy as np

import concourse.bass as bass
import concourse.mybir as mybir
from concourse.bass import AP, IndirectOffsetOnAxis
from concourse.bass_utils import run_bass_kernel_spmd

F32 = mybir.dt.float32
I32 = mybir.dt.int32
U16 = mybir.dt.uint16

B, C, NV, H, W = 2, 16, 25, 128, 128
A = 5
MIND, MAXD = -4, 4
D = MAXD - MIND + 1
NCORES = 8
NS = (B * C) // NCORES      # slices per core = 4

RPP = 32                    # image rows per partition
G = H // RPP                # row groups per tile = 4
FREE = RPP * W              # elems per partition per (view, slice) = 4096

X_V = H * W                 # input view stride (elems)
X_S = NV * X_V              # input slice stride

NTILES = NS * NV * D        # 900 tiles per core

RAWOFF = 0                  # 3-slot RAW column ring
NRAW = 3
BANKOFF = RAWOFF + NRAW * FREE
ZOFF = BANKOFF + 2 * 4 * FREE
ZLEN = 1024
PITCH = ZOFF + ZLEN         # 46080 elems = 180KB per partition

DNEG = [-4, -3, -2, -1]
DPOS = [1, 2, 3, 4]

# idx table layout (int32 block indices, partitions 0..79)
IDX_D0 = 0                  # 5 entries (cols 0..4)
IDX_BD = 8                  # 40 entries: (bank bk)*4 + slot
IPITCH = IDX_BD + 40


def _p0(a1, g=0, s=0):
    """Partition of (a1, g, s): g-major, s-minor within each a1 block."""
    return 16 * a1 + 4 * g + s


def _dlist(bk):
    return DNEG if bk % 2 == 0 else DPOS


def _rshift(d, a1):
    return d * (A // 2 - a1)


def _pad_schedule():
    """Start row S(t) for tile t = ((s*NV + v)*D + dt), with
    S(t) == r(t) (mod 32) and >=16 rows between tiles.

    Every view starts at a multiple of 32, and the 9 tiles within a
    view are laid out by a per-a1 template (r depends only on (a1, d)),
    so the view length V_LEN[a1] and slice length S_LEN are uniform
    across a2 and s by construction.
    """
    within = {}   # a1 -> list of 9 start rows (relative to view base)
    vlen = {}     # a1 -> padded view length (multiple of 32)
    for a1 in range(A):
        starts = []
        cur = 16
        for dt in range(D):
            r = _rshift(dt + MIND, a1) % 32
            start = cur + ((r - cur) % 32)
            starts.append(start)
            cur = start + H + 16
        within[a1] = starts
        vlen[a1] = cur + (-cur) % 32
    S = np.zeros(NTILES, np.int64)
    for s in range(NS):
        for v in range(NV):
            a1, a2 = v // A, v % A
            base = (
                s * sum(A * vlen[q] for q in range(A))
                + sum(A * vlen[q] for q in range(a1))
                + a2 * vlen[a1]
            )
            for dt in range(D):
                S[(s * NV + v) * D + dt] = base + within[a1][dt]
    v_len = [vlen[a1] for a1 in range(A)]
    s_len = sum(A * vlen[q] for q in range(A))
    return S, NS * s_len, v_len, s_len


S_TBL, TOTR, V_LEN, S_LEN = _pad_schedule()
TOT4K = TOTR // RPP


def make_idx_table() -> np.ndarray:
    """Host-precomputed 16KB-block dest indices (identical per core)."""
    idx = np.zeros((80, IPITCH), np.int32)
    for p in range(80):
        a1, g, s = p // 16, (p % 16) // 4, p % 4
        for col in range(A):
            t = (s * NV + a1 * A + col) * D + (0 - MIND)
            idx[p, IDX_D0 + col] = (S_TBL[t] + RPP * g) // RPP
        for bk in range(10):
            col = bk // 2
            for j, d in enumerate(_dlist(bk)):
                r = _rshift(d, a1)
                t = (s * NV + a1 * A + col) * D + (d - MIND)
                dest = S_TBL[t] + RPP * g - r
                assert dest % RPP == 0
                idx[p, IDX_BD + 4 * bk + j] = dest // RPP
    return idx


def make_rowsel() -> np.ndarray:
    """Row gather table: padded out rows -> logical output rows."""
    return (S_TBL[:, None] + np.arange(H)[None, :]).reshape(-1)


def _build_nc():
    nc = bass.Bass()
    x = nc.dram_tensor("x", [NS, NV, H, W], F32, kind="ExternalInput")
    idx = nc.dram_tensor("idx", [80, IPITCH], I32, kind="ExternalInput")
    out = nc.dram_tensor("out", [TOTR, W], F32, kind="ExternalOutput")

    # zero-band jobs (d, a1) with r != 0; batched over (a2, s) in one DMA
    zjobs = [
        (d, a1)
        for a1 in range(A)
        for d in DNEG + DPOS
        if _rshift(d, a1) != 0
    ]

    with (
        ExitStack() as stack,
        nc.sbuf_tensor([128, PITCH], F32) as buf,
        nc.sbuf_tensor([128, IPITCH], I32) as ibuf,
        nc.semaphore("isem") as isem,   # idx table loaded
        nc.semaphore("vsem") as vsem,   # staged banks (1/bank, DVE-ordered)
        nc.semaphore("zsem") as zsem,   # zeros region ready
        nc.semaphore("d0s") as d0s,     # d0 store completions (16/col)
        nc.semaphore("zsy") as zsy,     # sync-ring zero bands
        nc.semaphore("zsc") as zsc,     # scalar-ring zero bands
        nc.Block() as block,
    ):
        # per-column load sems (waited at full total 80 = 5 view DMAs);
        # per-bank-slot store sems (full totals, 64/bank use)
        lsc = [stack.enter_context(nc.semaphore(f"lsc{j}")) for j in range(A)]
        gs = [stack.enter_context(nc.semaphore(f"gs{i}")) for i in range(2)]

        @block.vector
        def _(vector):
            vector.memset(AP(buf, ZOFF, [[PITCH, 128], [1, ZLEN]]), 0.0)\
                .then_inc(zsem, 1)
            cur_col = None
            for bk in range(10):
                col = bk // 2
                if col != cur_col:
                    cur_col = col
                    vector.wait_ge(lsc[col], 80)
                if bk >= 2:
                    vector.wait_ge(gs[bk % 2], 64 * (bk // 2))
                so = BANKOFF + (bk % 2) * 4 * FREE
                raw = RAWOFF + (col % NRAW) * FREE
                for i, d in enumerate(_dlist(bk)):
                    c = d * (A // 2 - col)
                    n = W - abs(c)
                    src_off = raw + max(c, 0)
                    dst_off = so + i * FREE + max(-c, 0)
                    op = vector.tensor_copy(
                        out=AP(buf, dst_off, [[PITCH, 80], [W, RPP], [1, n]]
                               ).bitcast(U16),
                        in_=AP(buf, src_off, [[PITCH, 80], [W, RPP], [1, n]]
                               ).bitcast(U16),
                    )
                    if c != 0:
                        m_off = so + i * FREE + (W - c if c > 0 else 0)
                        op = vector.memset(
                            AP(buf, m_off,
                               [[PITCH, 80], [W, RPP], [1, abs(c)]]),
                            0.0,
                        )
                    if i == 3:
                        op.then_inc(vsem, 1)
            vector.wait_ge(gs[0], 320)
            vector.wait_ge(gs[1], 320)

        def load_col(gpsimd, col):
            for a1 in range(A):
                gpsimd.dma_start(
                    out=AP(buf, _p0(a1) * PITCH + RAWOFF + (col % NRAW) * FREE,
                           [[PITCH, 16], [1, FREE]]),
                    in_=AP(x, (a1 * A + col) * X_V,
                           [[FREE, G], [X_S, NS], [1, FREE]]),
                ).then_inc(lsc[col], 16)

        def scatter80(gpsimd, src_off, icol, sem):
            """One 16KB block per partition 0..79 -> idx[p, icol]."""
            return gpsimd.indirect_dma_start(
                out=AP(out, 0, [[FREE, TOT4K], [1, FREE]]),
                out_offset=IndirectOffsetOnAxis(
                    ap=AP(ibuf, icol, [[IPITCH, 80], [1, 1]]), axis=0
                ),
                in_=AP(buf, src_off, [[PITCH, 80], [1, FREE]]),
                in_offset=None,
            ).then_inc(sem, 16)

        @block.gpsimd
        def _(gpsimd):
            gpsimd.dma_start(
                out=AP(ibuf, 0, [[IPITCH, 80], [1, IPITCH]]),
                in_=AP(idx, 0, [[IPITCH, 80], [1, IPITCH]]),
            ).then_inc(isem, 16)
            for col in range(NRAW):
                load_col(gpsimd, col)
            gpsimd.wait_ge(isem, 16)
            for col in range(A):
                gpsimd.wait_ge(lsc[col], 80)
                scatter80(gpsimd, RAWOFF + (col % NRAW) * FREE,
                          IDX_D0 + col, d0s)
                for parity in range(2):
                    bk = 2 * col + parity
                    so = BANKOFF + (bk % 2) * 4 * FREE
                    gpsimd.wait_ge(vsem, bk + 1)
                    for j in range(4):
                        scatter80(gpsimd, so + j * FREE,
                                  IDX_BD + 4 * bk + j, gs[bk % 2])
                if col + NRAW < A:
                    gpsimd.wait_ge(d0s, 16 * (col + 1))
                    load_col(gpsimd, col + NRAW)
            gpsimd.wait_ge(d0s, 80)
            gpsimd.wait_ge(gs[0], 320)
            gpsimd.wait_ge(gs[1], 320)

        def zero_band(engine, d, a1, sem):
            r = _rshift(d, a1)
            t0 = (a1 * A) * D + (d - MIND)
            dst = int(S_TBL[t0]) + (H - r if r > 0 else 0)
            engine.dma_start(
                out=AP(out, dst * W,
                       [[V_LEN[a1] * W, A], [S_LEN * W, NS], [1, abs(r) * W]]),
                in_=AP(buf, ZOFF, [[PITCH, A * NS], [1, abs(r) * W]]),
            ).then_inc(sem, 16)

        @block.sync
        def _(sync):
            sync.wait_ge(zsem, 1)
            for d, a1 in zjobs[0::2]:
                zero_band(sync, d, a1, zsy)
            sync.wait_ge(zsy, 16 * len(zjobs[0::2]))

        @block.scalar
        def _(scalar):
            scalar.wait_ge(zsem, 1)
            for d, a1 in zjobs[1::2]:
                zero_band(scalar, d, a1, zsc)
            scalar.wait_ge(zsc, 16 * len(zjobs[1::2]))

    return nc


_NC = None
_IDX = None
_ROWSEL = None


def _get_nc():
    global _NC
    if _NC is None:
        _NC = _build_nc()
    return _NC


def _get_idx():
    global _IDX
    if _IDX is None:
        _IDX = make_idx_table()
    return _IDX


def _get_rowsel():
    global _ROWSEL
    if _ROWSEL is None:
        _ROWSEL = make_rowsel()
    return _ROWSEL


def kernel(x: np.ndarray) -> np.ndarray:
    assert x.shape == (B, C, NV, H, W), x.shape
    xs = np.ascontiguousarray(x.astype(np.float32, copy=False)).reshape(
        B * C, NV, H, W
    )
    tbl = _get_idx()
    sel = _get_rowsel()
    in_maps = [
        {"x": xs[NS * k : NS * (k + 1)], "idx": tbl} for k in range(NCORES)
    ]
    res = run_bass_kernel_spmd(_get_nc(), in_maps, core_ids=list(range(NCORES)))
    out = np.concatenate(
        [r["out"][sel].reshape(NS, NV, D, H, W) for r in res.results], axis=0
    )
    return out.reshape(B, C, NV, D, H, W)


# revision 3
# speedup vs baseline: 1.8826x; 1.0144x over previous
"""Trainium2 Bass kernel for the light-field disparity cost-volume build.

Input  x:   (2, 16, 25, 128, 128) f32  (b, c, n=angRes^2, h, w)
Output:     (2, 16, 25, 9, 128, 128) f32  (b, c, n, D, h, w)

out[b,c,(a1,a2),d,y,x] = x[b,c,(a1,a2), y + d*(2-a1), x + d*(2-a2)]
(zero outside the image), d in [-4, 4].

Pure data movement; sharding: 32 (b*c) slices split 4-per-core over 8
NeuronCores (data parallel).

v4 strategy (indirect scatter of whole partition blocks): the previous
design issued ~420 regular dma_starts per core; the Q7 SWDGE costs ~1us
per dma_start (994ns fixed + 0.34ns/descriptor) and the HWDGE rings
deal all their descriptors to SDMA engines 0-3 only, so the gpsimd
issue rate (~227us) and the engine 0-3 hot spot (~330us busy) capped
the kernel at ~361us.

Key HW facts (probed):
  - indirect_dma_start supports exactly ONE block per SBUF partition
    (block = the per-partition contiguous run of in_, idx[p] gives the
    DRAM block index; multi-block-per-partition offset tables scramble).
  - Issue cost ~1us per instruction regardless of descriptor count, and
    SWDGE round-robins descriptors over all 16 SDMA engines.

Design: the output tensor is written PADDED ([TOTR, 128] rows; host
un-pads with a row gather). Each (slice s, view v, disparity d) tile
gets a start row S(t) chosen so S(t) == r (mod 32) where r = d*(2-a1)
is the tile's row shift, with >=16 pad rows between tiles. Then every
one of the tile's 4 row-groups - including the partial edge group - can
be stored as a full 16KB partition block at 4096-elem-aligned dest
(32*g - r + S(t) is a multiple of 32 rows): the <=8 overhanging rows
land in the inter-tile padding. So per (column a2, d-sign bank, d) ONE
indirect_dma_start scatters all 80 partition blocks (5 a1 x 4 g x 4 s),
fulls and edges alike, with no partial-group descriptors at all. d=0
tiles are scattered straight from RAW the same way (one instruction
per column; also kills the old DRAM->DRAM d0 copy's extra 6.5MB read).

Per core: 1 idx load + 25 view loads + 5 d0 + 40 bank-store indirect
instructions on gpsimd (~70us of Q7), 16 tiny zero-band DMAs per HWDGE
ring, all store descriptors 16KB and evenly spread - leaving the HBM
write floor (~59MB at ~358GB/s plus 6.5MB of reads) as the limiter.

The pad schedule is periodic in (a1, d) (r doesn't depend on a2 or s),
so view stride V_LEN(a1) and slice stride S_LEN are uniform and the
zero bands keep the baseline's batched [[a2 x s]] regular-DMA shape.

DVE staging: per column a2, two 4-slot banks (d<0, d>0) of
column-shifted copies (u16-bitcast tensor_copy + margin memsets; the
a2=2 column is a plain copy); banks double-buffer against their
stores; RAW columns rotate through a 3-slot ring so loads run 3
columns ahead.
"""

from contextlib import ExitStack

import numpy as np

import concourse.bass as bass
import concourse.mybir as mybir
from concourse.bass import AP, IndirectOffsetOnAxis
from concourse.bass_utils import run_bass_kernel_spmd

F32 = mybir.dt.float32
I32 = mybir.dt.int32
U16 = mybir.dt.uint16

B, C, NV, H, W = 2, 16, 25, 128, 128
A = 5
MIND, MAXD = -4, 4
D = MAXD - MIND + 1
NCORES = 8
NS = (B * C) // NCORES      # slices per core = 4

RPP = 32                    # image rows per partition
G = H // RPP                # row groups per tile = 4
FREE = RPP * W              # elems per partition per (view, slice) = 4096

X_V = H * W                 # input view stride (elems)
X_S = NV * X_V              # input slice stride

NTILES = NS * NV * D        # 900 tiles per core

RAWOFF = 0                  # 3-slot RAW column ring
NRAW = 3
BANKOFF = RAWOFF + NRAW * FREE
ZOFF = BANKOFF + 2 * 4 * FREE
ZLEN = 1024
PITCH = ZOFF + ZLEN         # 46080 elems = 180KB per partition

DNEG = [-4, -3, -2, -1]
DPOS = [1, 2, 3, 4]

# idx table layout (int32 block indices, partitions 0..79)
IDX_D0 = 0                  # 5 entries (cols 0..4)
IDX_BD = 8                  # 40 entries: (bank bk)*4 + slot
IPITCH = IDX_BD + 40


def _p0(a1, g=0, s=0):
    """Partition of (a1, g, s): g-major, s-minor within each a1 block."""
    return 16 * a1 + 4 * g + s


def _dlist(bk):
    return DNEG if bk % 2 == 0 else DPOS


def _rshift(d, a1):
    return d * (A // 2 - a1)


def _pad_schedule():
    """Start row S(t) for tile t = ((s*NV + v)*D + dt), with
    S(t) == r(t) (mod 32) and >=16 rows between tiles.

    Every view starts at a multiple of 32, and the 9 tiles within a
    view are laid out by a per-a1 template (r depends only on (a1, d)),
    so the view length V_LEN[a1] and slice length S_LEN are uniform
    across a2 and s by construction.
    """
    within = {}   # a1 -> list of 9 start rows (relative to view base)
    vlen = {}     # a1 -> padded view length (multiple of 32)
    for a1 in range(A):
        starts = []
        cur = 16
        for dt in range(D):
            r = _rshift(dt + MIND, a1) % 32
            start = cur + ((r - cur) % 32)
            starts.append(start)
            cur = start + H + 16
        within[a1] = starts
        vlen[a1] = cur + (-cur) % 32
    S = np.zeros(NTILES, np.int64)
    for s in range(NS):
        for v in range(NV):
            a1, a2 = v // A, v % A
            base = (
                s * sum(A * vlen[q] for q in range(A))
                + sum(A * vlen[q] for q in range(a1))
                + a2 * vlen[a1]
            )
            for dt in range(D):
                S[(s * NV + v) * D + dt] = base + within[a1][dt]
    v_len = [vlen[a1] for a1 in range(A)]
    s_len = sum(A * vlen[q] for q in range(A))
    return S, NS * s_len, v_len, s_len


S_TBL, TOTR, V_LEN, S_LEN = _pad_schedule()
TOT4K = TOTR // RPP


def make_idx_table() -> np.ndarray:
    """Host-precomputed 16KB-block dest indices (identical per core)."""
    idx = np.zeros((80, IPITCH), np.int32)
    for p in range(80):
        a1, g, s = p // 16, (p % 16) // 4, p % 4
        for col in range(A):
            t = (s * NV + a1 * A + col) * D + (0 - MIND)
            idx[p, IDX_D0 + col] = (S_TBL[t] + RPP * g) // RPP
        for bk in range(10):
            col = bk // 2
            for j, d in enumerate(_dlist(bk)):
                r = _rshift(d, a1)
                t = (s * NV + a1 * A + col) * D + (d - MIND)
                dest = S_TBL[t] + RPP * g - r
                assert dest % RPP == 0
                idx[p, IDX_BD + 4 * bk + j] = dest // RPP
    return idx


def make_rowsel() -> np.ndarray:
    """Row gather table: padded out rows -> logical output rows."""
    return (S_TBL[:, None] + np.arange(H)[None, :]).reshape(-1)


def _build_nc():
    nc = bass.Bass()
    x = nc.dram_tensor("x", [NS, NV, H, W], F32, kind="ExternalInput")
    idx = nc.dram_tensor("idx", [80, IPITCH], I32, kind="ExternalInput")
    out = nc.dram_tensor("out", [TOTR, W], F32, kind="ExternalOutput")

    # zero-band jobs (d, a1) with r != 0; batched over (a2, s) in one DMA
    zjobs = [
        (d, a1)
        for a1 in range(A)
        for d in DNEG + DPOS
        if _rshift(d, a1) != 0
    ]

    with (
        ExitStack() as stack,
        nc.sbuf_tensor([128, PITCH], F32) as buf,
        nc.sbuf_tensor([128, IPITCH], I32) as ibuf,
        nc.semaphore("isem") as isem,   # idx table loaded
        nc.semaphore("vsem") as vsem,   # staged banks (1/bank, DVE-ordered)
        nc.semaphore("zsem") as zsem,   # zeros region ready
        nc.semaphore("d0s") as d0s,     # d0 store completions (16/col)
        nc.semaphore("zsy") as zsy,     # sync-ring zero bands
        nc.semaphore("zsc") as zsc,     # scalar-ring zero bands
        nc.Block() as block,
    ):
        # per-column load sems (waited at full total 80 = 5 view DMAs);
        # per-bank-slot store sems (full totals, 64/bank use)
        lsc = [stack.enter_context(nc.semaphore(f"lsc{j}")) for j in range(A)]
        gs = [stack.enter_context(nc.semaphore(f"gs{i}")) for i in range(2)]

        @block.vector
        def _(vector):
            vector.memset(AP(buf, ZOFF, [[PITCH, 128], [1, ZLEN]]), 0.0)\
                .then_inc(zsem, 1)
            cur_col = None
            for bk in range(10):
                col = bk // 2
                if col != cur_col:
                    cur_col = col
                    vector.wait_ge(lsc[col], 80)
                if bk >= 2:
                    vector.wait_ge(gs[bk % 2], 64 * (bk // 2))
                so = BANKOFF + (bk % 2) * 4 * FREE
                raw = RAWOFF + (col % NRAW) * FREE
                for i, d in enumerate(_dlist(bk)):
                    c = d * (A // 2 - col)
                    n = W - abs(c)
                    src_off = raw + max(c, 0)
                    dst_off = so + i * FREE + max(-c, 0)
                    op = vector.tensor_copy(
                        out=AP(buf, dst_off, [[PITCH, 80], [W, RPP], [1, n]]
                               ).bitcast(U16),
                        in_=AP(buf, src_off, [[PITCH, 80], [W, RPP], [1, n]]
                               ).bitcast(U16),
                    )
                    if c != 0:
                        m_off = so + i * FREE + (W - c if c > 0 else 0)
                        op = vector.memset(
                            AP(buf, m_off,
                               [[PITCH, 80], [W, RPP], [1, abs(c)]]),
                            0.0,
                        )
                    if i == 3:
                        op.then_inc(vsem, 1)
            vector.wait_ge(gs[0], 320)
            vector.wait_ge(gs[1], 320)

        def load_col(gpsimd, col):
            for a1 in range(A):
                gpsimd.dma_start(
                    out=AP(buf, _p0(a1) * PITCH + RAWOFF + (col % NRAW) * FREE,
                           [[PITCH, 16], [1, FREE]]),
                    in_=AP(x, (a1 * A + col) * X_V,
                           [[FREE, G], [X_S, NS], [1, FREE]]),
                ).then_inc(lsc[col], 16)

        def scatter80(gpsimd, src_off, icol, sem):
            """One 16KB block per partition 0..79 -> idx[p, icol]."""
            return gpsimd.indirect_dma_start(
                out=AP(out, 0, [[FREE, TOT4K], [1, FREE]]),
                out_offset=IndirectOffsetOnAxis(
                    ap=AP(ibuf, icol, [[IPITCH, 80], [1, 1]]), axis=0
                ),
                in_=AP(buf, src_off, [[PITCH, 80], [1, FREE]]),
                in_offset=None,
            ).then_inc(sem, 16)

        @block.gpsimd
        def _(gpsimd):
            gpsimd.dma_start(
                out=AP(ibuf, 0, [[IPITCH, 80], [1, IPITCH]]),
                in_=AP(idx, 0, [[IPITCH, 80], [1, IPITCH]]),
            ).then_inc(isem, 16)
            for col in range(NRAW):
                load_col(gpsimd, col)
            gpsimd.wait_ge(isem, 16)
            for col in range(A):
                gpsimd.wait_ge(lsc[col], 80)
                # d=0 tiles: DRAM->DRAM (no SBUF ports), 8KB descriptors;
                # padded dest strides are uniform over s for fixed view
                for a1 in range(A):
                    t0 = ((a1 * A + col) * D) + (0 - MIND)
                    gpsimd.dma_start(
                        out=AP(out, int(S_TBL[t0]) * W,
                               [[S_LEN * W, NS], [1, X_V]]),
                        in_=AP(x, (a1 * A + col) * X_V,
                               [[X_S, NS], [1, X_V]]),
                        max_dma_last_dim=8192,
                    ).then_inc(d0s, 16)
                for parity in range(2):
                    bk = 2 * col + parity
                    so = BANKOFF + (bk % 2) * 4 * FREE
                    gpsimd.wait_ge(vsem, bk + 1)
                    for j in range(4):
                        scatter80(gpsimd, so + j * FREE,
                                  IDX_BD + 4 * bk + j, gs[bk % 2])
                if col + NRAW < A:
                    load_col(gpsimd, col + NRAW)
            gpsimd.wait_ge(d0s, 16 * 25)
            gpsimd.wait_ge(gs[0], 320)
            gpsimd.wait_ge(gs[1], 320)

        def zero_band(engine, d, a1, sem):
            r = _rshift(d, a1)
            t0 = (a1 * A) * D + (d - MIND)
            dst = int(S_TBL[t0]) + (H - r if r > 0 else 0)
            engine.dma_start(
                out=AP(out, dst * W,
                       [[V_LEN[a1] * W, A], [S_LEN * W, NS], [1, abs(r) * W]]),
                in_=AP(buf, ZOFF, [[PITCH, A * NS], [1, abs(r) * W]]),
            ).then_inc(sem, 16)

        @block.sync
        def _(sync):
            sync.wait_ge(zsem, 1)
            for d, a1 in zjobs[0::2]:
                zero_band(sync, d, a1, zsy)
            sync.wait_ge(zsy, 16 * len(zjobs[0::2]))

        @block.scalar
        def _(scalar):
            scalar.wait_ge(zsem, 1)
            for d, a1 in zjobs[1::2]:
                zero_band(scalar, d, a1, zsc)
            scalar.wait_ge(zsc, 16 * len(zjobs[1::2]))

    return nc


_NC = None
_IDX = None
_ROWSEL = None


def _get_nc():
    global _NC
    if _NC is None:
        _NC = _build_nc()
    return _NC


def _get_idx():
    global _IDX
    if _IDX is None:
        _IDX = make_idx_table()
    return _IDX


def _get_rowsel():
    global _ROWSEL
    if _ROWSEL is None:
        _ROWSEL = make_rowsel()
    return _ROWSEL


def kernel(x: np.ndarray) -> np.ndarray:
    assert x.shape == (B, C, NV, H, W), x.shape
    xs = np.ascontiguousarray(x.astype(np.float32, copy=False)).reshape(
        B * C, NV, H, W
    )
    tbl = _get_idx()
    sel = _get_rowsel()
    in_maps = [
        {"x": xs[NS * k : NS * (k + 1)], "idx": tbl} for k in range(NCORES)
    ]
    res = run_bass_kernel_spmd(_get_nc(), in_maps, core_ids=list(range(NCORES)))
    out = np.concatenate(
        [r["out"][sel].reshape(NS, NV, D, H, W) for r in res.results], axis=0
    )
    return out.reshape(B, C, NV, D, H, W)
